# revision 21
# baseline (speedup 1.0000x reference)
"""Trainium2 Bass kernel for nn_InvNet_5214090297566 (retrieval_knn).

Strategy (class-sharded tensor parallel over the memory bank, 8 cores):
  - The memory bank em is the only large payload; it ships NATURAL layout
    as fp8_e4m3 (one plain host cast, no host transpose). fp8 is transport
    only: the PE transposes the natural tiles (fp8 transpose mode, step-2
    PSUM out) and the ACT drain converts to fp16, so all matmul arithmetic
    is fp16 x fp16 -> fp32. Quantizing em to fp8 shifts the reference loss
    by only ~2e-5 (measured) — far below tolerance.
  - Feature windows are 128-wide (PE transpose block). A split's tail
    (77/75 features) uses a window ending at the split boundary with the
    input rows above the tail zeroed, so the duplicated em rows contribute
    nothing to the matmul.
  - Target-class rows are gathered on host (pure indexing) and shipped
    replicated; every core computes the full per-split target dots locally
    (no em natural copy on device, no indirect gather, no cross-core sum).
  - Each core: for each 512-class chunk, fp16 matmuls produce per-split
    sims (PSUM) and the full sim (PSUM, accumulated over the same
    feature windows).
  - Packed-key trick: key = fp16(splitdot + 0.5) + (fulldot+0.25)*2^-12*(511/512).
    The fp16 cast quantizes the ranking value; the fulldot payload rides in
    the low mantissa bits (below half the fp16 ulp), so a single max8 pass
    yields top-8 candidates carrying both the ranking value and the full-sim
    value (recovered later by re-casting to fp16 and subtracting).
  - LSE partials via ACT exp+accumulate straight from PSUM (no max needed:
    sim/beta <= ~4 for unit-norm rows).
  - One AllGather of an 81-float-per-row blob (80 keys + 1 sumexp); every
    core redundantly merges (global 6th-largest key per (split,row), masked
    sums, final scalar).
  - Runner: the jax.jit(shard_map(...)) wrapper around the Bass custom call
    is built once and cached; per call, em shards are cast+device_put
    per-device in a pipelined loop (casts overlap in-flight transfers),
    then assembled with make_array_from_single_device_arrays.
"""

import os
os.environ.setdefault("JAX_PLATFORMS", "axon,cpu")

from contextlib import ExitStack

import numpy as np

import concourse.bacc as bacc
import concourse.bass_isa as bass_isa
import concourse.masks as masks
import concourse.mybir as mybir
import concourse.tile as tile

NCORES = 8
B = 256
C = 32768
F = 2048
CLOC = C // NCORES          # 4096 classes per core
NSPLITS = 10
STEP = -(-F // NSPLITS)     # 205
KNN = 6
ISCALE = 20.0               # 1/BETA
NCHUNK = 8                  # class chunks per core
CW = 512                    # chunk width (classes)
NT = 2                      # batch tiles of 128

# fp16 packing constants
HBIAS = 0.5
FQ_SCALE = float((2.0 ** -12) * (511.0 / 512.0))
FQ_OFF = float(0.25 * (2.0 ** -12) * (511.0 / 512.0))
REC_SCALE = float((2.0 ** 12) * (512.0 / 511.0))

# feature windows: per split s, two 128-wide windows
#   W1 = [205s, 205s+128)                 all rows valid (off=0)
#   W2 = [min(205(s+1),F)-128, ...+128)   rows [off:128) valid, off = overlap
# with the input rows [0:off) zeroed so duplicated em rows contribute 0.
WINDOWS = []
for s in range(NSPLITS):
    lo = STEP * s
    hi = min(STEP * (s + 1), F)
    WINDOWS.append((lo, 0))
    w2lo = hi - 128
    WINDOWS.append((w2lo, (lo + 128) - w2lo))

GP_KEY_SPLITS = 8  # splits whose packed key builds on gpsimd (rest on DVE)

F32 = mybir.dt.float32
F16 = mybir.dt.float16
F8 = mybir.dt.float8e4
AF = mybir.ActivationFunctionType
OP = mybir.AluOpType

SW = 81  # stage width: 80 keys + 1 sumexp


def _build(knn_on: bool):
    nc = bacc.Bacc("TRN2", target_bir_lowering=False, debug=False,
                   num_devices=NCORES)

    em_n = nc.dram_tensor("em_n", [CLOC, F], F8, kind="ExternalInput")
    # inp_t/inp_n/tgt_n ship SHARDED as raw bytes (192 fp8-rows per core) and
    # are broadcast on-device by an AllGather, instead of 8x replicated host
    # transfers.  SMROWS rows x F bytes per core; 3 x 1MiB tensors total.
    SMROWS = (2 * F * B * 3) // (F * NCORES)   # 192
    sm_in = nc.dram_tensor("sm_in", [SMROWS, F], F8, kind="ExternalInput")
    out_loss = nc.dram_tensor("loss", [1, 1], F32, kind="ExternalOutput")

    sm_st = nc.dram_tensor("sm_st", [SMROWS, F], F8, kind="Internal")
    sm_g = nc.dram_tensor("sm_g", [NCORES, SMROWS, F], F8, kind="Internal",
                          addr_space="Shared")
    stage = nc.dram_tensor("stage", [NT, 128, SW], F32, kind="Internal")
    gath = nc.dram_tensor("gath", [NCORES, NT, 128, SW], F32, kind="Internal",
                          addr_space="Shared")

    with tile.TileContext(nc) as tc, ExitStack() as ctx:
        singles = ctx.enter_context(tc.tile_pool(name="singles", bufs=1))
        slabs = ctx.enter_context(tc.tile_pool(name="slabs", bufs=2))
        work = ctx.enter_context(tc.tile_pool(name="work", bufs=3))
        keyp = ctx.enter_context(tc.tile_pool(name="keyp", bufs=4))
        big1 = ctx.enter_context(tc.tile_pool(name="big1", bufs=1))
        psum_f = ctx.enter_context(tc.tile_pool(name="psf", bufs=2, space="PSUM"))
        psum_s = ctx.enter_context(tc.tile_pool(name="pss", bufs=3, space="PSUM"))
        psum_t = ctx.enter_context(tc.tile_pool(name="pst", bufs=2, space="PSUM"))

        # ---- broadcast the sharded small tensors, build bitcast views ----
        # (collectives cannot read IO tensors; bounce through Internal dram)
        nc.sync.dma_start(out=sm_st[:, :], in_=sm_in[:, :])
        nc.gpsimd.collective_compute(
            "AllGather", OP.bypass,
            replica_groups=[list(range(NCORES))],
            ins=[sm_st[:, :]], outs=[sm_g[:, :, :]])
        # Tile does not order DMA reads after collective writes (only the
        # collective's input writers are annotated) — without this barrier
        # the sm_g reads race the gather and read stale bytes.
        tc.strict_bb_all_engine_barrier()
        sm_flat = sm_g[:, :, :].rearrange("a b c -> (a b c)")
        NB = 2 * F * B  # bytes per small tensor
        inp_t = sm_flat[0:NB].bitcast(F16).rearrange("(r c) -> r c", c=B)
        inp_n = sm_flat[NB:2 * NB].bitcast(F16).rearrange("(r c) -> r c", c=F)
        tgt_n = sm_flat[2 * NB:3 * NB].bitcast(F16).rearrange("(r c) -> r c",
                                                              c=F)

        # ---- persistent tiles ----
        id8 = singles.tile([128, 128], F8, tag="id8")
        masks.make_identity(nc, id8[:])

        in_slab = singles.tile([128, 20, B], F16, tag="in_slab")
        nc.vector.memset(in_slab[:], 0.0)
        for j, (wlo, off) in enumerate(WINDOWS):
            nc.sync.dma_start(out=in_slab[off:128, j, :],
                              in_=inp_t[wlo + off:wlo + 128, :])

        cand = {}
        for t in range(NT):
            for s in range(NSPLITS):
                cand[(t, s)] = singles.tile([128, NCHUNK * 8], F32,
                                            tag=f"cand{t}_{s}",
                                            name=f"cand{t}_{s}")
        se_cols = [singles.tile([128, NCHUNK], F32, tag=f"se{t}", name=f"se{t}")
                   for t in range(NT)]
        stage_sb = [singles.tile([128, SW], F32, tag=f"stage{t}", name=f"stg{t}")
                    for t in range(NT)]
        td_loc = [singles.tile([128, NSPLITS], F32, tag=f"td{t}", name=f"td{t}")
                  for t in range(NT)]
        tf_loc = [singles.tile([128, 1], F32, tag=f"tf{t}", name=f"tf{t}")
                  for t in range(NT)]

        # ---- target-class dots (local, from host-gathered rows) ----
        for t in range(NT):
            bsl = slice(t * 128, (t + 1) * 128)
            emt = big1.tile([128, F], F16, tag="emt")
            nc.sync.dma_start(out=emt[:], in_=tgt_n[bsl, :])
            inpn = big1.tile([128, F], F16, tag="inpn")
            nc.sync.dma_start(out=inpn[:], in_=inp_n[bsl, :])
            prod = big1.tile([128, F], F32, tag="prod")
            nc.vector.tensor_tensor(out=prod[:], in0=emt[:], in1=inpn[:],
                                    op=OP.mult)
            nc.vector.reduce_sum(
                out=td_loc[t][:, 0:9],
                in_=prod[:, 0:9 * STEP].rearrange("p (s w) -> p s w", s=9),
                axis=mybir.AxisListType.X)
            nc.vector.reduce_sum(out=td_loc[t][:, 9:10], in_=prod[:, 9 * STEP:F],
                                 axis=mybir.AxisListType.X)
            nc.vector.reduce_sum(out=tf_loc[t][:], in_=td_loc[t][:],
                                 axis=mybir.AxisListType.X)

        # ---- main streaming loops ----
        for c in range(NCHUNK):
            c0 = c * CW
            # natural-layout fp8 slab: 4 class-subtiles of 128 x all features
            nat8 = slabs.tile([128, 4, F], F8, tag="nat8")
            for q in range(4):
                nc.sync.dma_start(out=nat8[:, q, :],
                                  in_=em_n[c0 + q * 128:c0 + (q + 1) * 128, :])
            # PE-transpose each 128x128 block (fp8, step-2 PSUM out), ACT
            # drain converts to fp16 feature-major em_slab
            em_slab = slabs.tile([128, 20, CW], F16, tag="em_slab")
            for j, (wlo, off) in enumerate(WINDOWS):
                for q in range(4):
                    pst = psum_t.tile([128, 128, 2], F8, tag="pst")
                    nc.tensor.transpose(pst[:, :, 0],
                                        nat8[:, q, wlo:wlo + 128], id8[:])
                    nc.scalar.activation(
                        out=em_slab[:, j, q * 128:(q + 1) * 128],
                        in_=pst[:, :, 0], func=AF.Copy, scale=1.0)
            for t in range(NT):
                bsl = slice(t * 128, (t + 1) * 128)
                # full-sim accumulation over all 20 windows (padded rows are 0)
                fbank = psum_f.tile([128, CW], F32, tag="fbank")
                for j in range(20):
                    nc.tensor.matmul(out=fbank[:], lhsT=in_slab[:, j, bsl],
                                     rhs=em_slab[:, j, :],
                                     start=(j == 0), stop=(j == 19))
                # LSE partial: sum(exp(20*fulldot)) for this chunk
                junk = work.tile([128, CW], F32, tag="junk")
                nc.scalar.activation(out=junk[:], in_=fbank[:], func=AF.Exp,
                                     scale=ISCALE,
                                     accum_out=se_cols[t][:, c:c + 1])
                if knn_on:
                    # fq payload from full sim
                    fq = work.tile([128, CW], F32, tag="fq")
                    nc.scalar.activation(out=fq[:], in_=fbank[:], func=AF.Copy,
                                         scale=FQ_SCALE, bias=FQ_OFF)
                    # per-split sims + packed keys + top8
                    for s in range(NSPLITS):
                        sbank = psum_s.tile([128, CW], F32, tag="sbank")
                        for jj in (2 * s, 2 * s + 1):
                            nc.tensor.matmul(out=sbank[:],
                                             lhsT=in_slab[:, jj, bsl],
                                             rhs=em_slab[:, jj, :],
                                             start=(jj == 2 * s),
                                             stop=(jj == 2 * s + 1))
                        h16 = keyp.tile([128, CW], F16, tag="h16")
                        nc.scalar.activation(out=h16[:], in_=sbank[:],
                                             func=AF.Copy, scale=1.0,
                                             bias=HBIAS)
                        key = keyp.tile([128, CW], F32, tag="key")
                        eng = nc.gpsimd if s < GP_KEY_SPLITS else nc.vector
                        eng.tensor_tensor(out=key[:], in0=h16[:], in1=fq[:],
                                          op=OP.add)
                        nc.vector.max(out=cand[(t, s)][:, c * 8:(c + 1) * 8],
                                      in_=key[:])

        # ---- reduce LSE partials + core-level top8 into stage ----
        for t in range(NT):
            nc.vector.reduce_sum(out=stage_sb[t][:, 80:81], in_=se_cols[t][:],
                                 axis=mybir.AxisListType.X)
            if knn_on:
                for s in range(NSPLITS):
                    nc.vector.max(out=stage_sb[t][:, s * 8:(s + 1) * 8],
                                  in_=cand[(t, s)][:])
            nc.sync.dma_start(out=stage[t, :, :], in_=stage_sb[t][:])

        # ---- all-gather ----
        nc.gpsimd.collective_compute(
            "AllGather", OP.bypass,
            replica_groups=[list(range(NCORES))],
            ins=[stage[:, :, :]], outs=[gath[:, :, :, :]])
        tc.strict_bb_all_engine_barrier()

        # ---- final merge (redundant on every core) ----
        rl_tot = singles.tile([128, 1], F32, tag="rl_tot")
        nc.vector.memset(rl_tot[:], 0.0)
        for t in range(NT):
            # global sumexp -> LSE
            se8 = work.tile([128, NCORES], F32, tag="se8")
            nc.sync.dma_start(
                out=se8[:],
                in_=gath[:, t, :, 80:81].rearrange("c p w -> p c w"))
            zt = work.tile([128, 1], F32, tag="zt")
            nc.vector.reduce_sum(out=zt[:], in_=se8[:],
                                 axis=mybir.AxisListType.X)
            lse = work.tile([128, 1], F32, tag="lse")
            nc.scalar.activation(out=lse[:], in_=zt[:], func=AF.Ln)
            tfull = tf_loc[t][:, 0:1]
            # logp_t = 20*fulldot_t - LSE
            logpt = work.tile([128, 1], F32, tag="logpt")
            nc.vector.tensor_scalar(out=logpt[:], in0=tfull, scalar1=ISCALE,
                                    scalar2=None, op0=OP.mult)
            nc.vector.tensor_tensor(out=logpt[:], in0=logpt[:], in1=lse[:],
                                    op=OP.subtract)
            rl = work.tile([128, 1], F32, tag="rl")
            if not knn_on:
                nc.vector.tensor_scalar(out=rl[:], in0=logpt[:], scalar1=-1.0,
                                        scalar2=None, op0=OP.mult)
            else:
                knn_cols = work.tile([128, NSPLITS], F32, tag="knncols")
                cnt_cols = work.tile([128, NSPLITS], F32, tag="cntcols")
                v6_cols = work.tile([128, NSPLITS], F32, tag="v6cols")
                for s in range(NSPLITS):
                    k64 = keyp.tile([128, NCORES * 8], F32, tag="k64")
                    nc.sync.dma_start(
                        out=k64[:],
                        in_=gath[:, t, :, s * 8:(s + 1) * 8]
                        .rearrange("c p w -> p c w"))
                    m8 = work.tile([128, 8], F32, tag="m8")
                    nc.vector.max(out=m8[:], in_=k64[:])
                    nc.vector.tensor_copy(out=v6_cols[:, s:s + 1],
                                          in_=m8[:, 5:6])
                    # decode fulldot payload
                    k16 = work.tile([128, NCORES * 8], F16, tag="k16")
                    nc.vector.tensor_copy(out=k16[:], in_=k64[:])
                    fd = work.tile([128, NCORES * 8], F32, tag="fd")
                    nc.vector.tensor_tensor(out=fd[:], in0=k64[:], in1=k16[:],
                                            op=OP.subtract)
                    nc.vector.tensor_scalar(out=fd[:], in0=fd[:],
                                            scalar1=REC_SCALE, scalar2=-0.25,
                                            op0=OP.mult, op1=OP.add)
                    mask = work.tile([128, NCORES * 8], F32, tag="mask")
                    nc.vector.tensor_scalar(out=mask[:], in0=k64[:],
                                            scalar1=v6_cols[:, s:s + 1],
                                            scalar2=None, op0=OP.is_ge)
                    nc.vector.reduce_sum(out=cnt_cols[:, s:s + 1], in_=mask[:],
                                         axis=mybir.AxisListType.X)
                    nc.vector.scalar_tensor_tensor(
                        out=fd[:], in0=mask[:], scalar=ISCALE, in1=fd[:],
                        op0=OP.mult, op1=OP.mult,
                        accum_out=knn_cols[:, s:s + 1])
                # target keys, same packing construction
                th = work.tile([128, NSPLITS], F32, tag="th")
                nc.vector.tensor_scalar(out=th[:], in0=td_loc[t][:],
                                        scalar1=HBIAS, scalar2=None, op0=OP.add)
                th16 = work.tile([128, NSPLITS], F16, tag="th16")
                nc.vector.tensor_copy(out=th16[:], in_=th[:])
                tfq = work.tile([128, 1], F32, tag="tfq")
                nc.vector.tensor_scalar(out=tfq[:], in0=tfull, scalar1=FQ_SCALE,
                                        scalar2=FQ_OFF, op0=OP.mult, op1=OP.add)
                tkey = work.tile([128, NSPLITS], F32, tag="tkey")
                nc.vector.tensor_scalar(out=tkey[:], in0=th16[:],
                                        scalar1=tfq[:, 0:1], scalar2=None,
                                        op0=OP.add)
                tmask = work.tile([128, NSPLITS], F32, tag="tmask")
                nc.vector.tensor_tensor(out=tmask[:], in0=tkey[:],
                                        in1=v6_cols[:], op=OP.is_ge)
                # rowloss = -10*logp_t - (A - LSE*Cc - logp_t*Tm)/6
                A = work.tile([128, 1], F32, tag="A")
                nc.vector.reduce_sum(out=A[:], in_=knn_cols[:],
                                     axis=mybir.AxisListType.X)
                Cc = work.tile([128, 1], F32, tag="Cc")
                nc.vector.reduce_sum(out=Cc[:], in_=cnt_cols[:],
                                     axis=mybir.AxisListType.X)
                Tm = work.tile([128, 1], F32, tag="Tm")
                nc.vector.reduce_sum(out=Tm[:], in_=tmask[:],
                                     axis=mybir.AxisListType.X)
                u1 = work.tile([128, 1], F32, tag="u1")
                nc.vector.tensor_tensor(out=u1[:], in0=lse[:], in1=Cc[:],
                                        op=OP.mult)
                u2 = work.tile([128, 1], F32, tag="u2")
                nc.vector.tensor_tensor(out=u2[:], in0=logpt[:], in1=Tm[:],
                                        op=OP.mult)
                nc.vector.tensor_tensor(out=A[:], in0=A[:], in1=u1[:],
                                        op=OP.subtract)
                nc.vector.tensor_tensor(out=A[:], in0=A[:], in1=u2[:],
                                        op=OP.subtract)
                nc.vector.tensor_scalar(out=rl[:], in0=logpt[:],
                                        scalar1=-float(NSPLITS), scalar2=None,
                                        op0=OP.mult)
                nc.vector.tensor_scalar(out=A[:], in0=A[:],
                                        scalar1=-1.0 / KNN, scalar2=None,
                                        op0=OP.mult)
                nc.vector.tensor_tensor(out=rl[:], in0=rl[:], in1=A[:],
                                        op=OP.add)
            nc.vector.tensor_tensor(out=rl_tot[:], in0=rl_tot[:], in1=rl[:],
                                    op=OP.add)

        # partition sum -> scalar
        pr = singles.tile([128, 1], F32, tag="pr")
        nc.gpsimd.partition_all_reduce(out_ap=pr[:], in_ap=rl_tot[:],
                                       channels=128,
                                       reduce_op=bass_isa.ReduceOp.add)
        res = singles.tile([1, 1], F32, tag="res")
        denom = float(NSPLITS * B) if knn_on else float(B)
        nc.vector.tensor_scalar(out=res[:], in0=pr[0:1, 0:1],
                                scalar1=1.0 / denom, scalar2=None, op0=OP.mult)
        nc.sync.dma_start(out=out_loss[:, :], in_=res[:])

    nc.finalize()
    return nc


class _Runner:
    """jit(shard_map(bass_exec)) built once; per call, em shards are
    cast+put per-device (pipelined), smalls staged sharded, then invoked."""

    def __init__(self, knn_on: bool):
        import jax
        from jax.sharding import Mesh, NamedSharding, PartitionSpec
        from jax.experimental.shard_map import shard_map
        from concourse.bass2jax import (_bass_exec_p, install_neuronx_cc_hook,
                                        partition_id_tensor)

        self.jax = jax
        import jax.numpy as jnp
        import ml_dtypes
        self.cast8 = jax.jit(lambda a: a.astype(jnp.float8_e4m3))
        self.f8np = ml_dtypes.float8_e4m3
        nc = _build(knn_on)
        install_neuronx_cc_hook()

        partition_name = (nc.partition_id_tensor.name
                          if nc.partition_id_tensor else None)
        in_names, out_names, out_avals = [], [], []
        for alloc in nc.m.functions[0].allocations:
            if not isinstance(alloc, mybir.MemoryLocationSet):
                continue
            name = alloc.memorylocations[0].name
            if alloc.kind == "ExternalInput":
                if name != partition_name:
                    in_names.append(name)
            elif alloc.kind == "ExternalOutput":
                out_names.append(name)
                out_avals.append(jax.core.ShapedArray(
                    tuple(alloc.tensor_shape), mybir.dt.np(alloc.dtype)))
        self.in_names = in_names
        self.out_names = out_names
        self.out_shapes = [(tuple(a.shape), a.dtype) for a in out_avals]
        n_params = len(in_names)
        n_outs = len(out_avals)
        all_in_names = list(in_names) + list(out_names)
        if partition_name is not None:
            all_in_names.append(partition_name)

        def _body(*args):
            operands = list(args)
            if partition_name is not None:
                operands.append(partition_id_tensor())
            return tuple(_bass_exec_p.bind(
                *operands, out_avals=tuple(out_avals),
                in_names=tuple(all_in_names), out_names=tuple(out_names),
                lowering_input_output_aliases=(),
                sim_require_finite=True, sim_require_nnan=True, nc=nc))

        self.devices = list(jax.devices()[:NCORES])
        mesh = Mesh(np.asarray(self.devices), ("core",))
        self.sharding = NamedSharding(mesh, PartitionSpec("core"))
        in_specs = (PartitionSpec("core"),) * (n_params + n_outs)
        out_specs = (PartitionSpec("core"),) * n_outs
        self.fn = jax.jit(
            shard_map(_body, mesh=mesh, in_specs=in_specs,
                      out_specs=out_specs, check_rep=False),
            donate_argnums=tuple(range(n_params, n_params + n_outs)),
            keep_unused=True)

    def __call__(self, dev_inputs: dict):
        jax = self.jax
        zeros = [np.zeros((NCORES * s[0], *s[1:]), d)
                 for (s, d) in self.out_shapes]
        out = self.fn(*[dev_inputs[n] for n in self.in_names], *zeros)
        jax.block_until_ready(out)
        i = self.out_names.index("loss")
        return np.asarray(out[i]).reshape(NCORES, 1, 1)[0]


_RUNNERS = {}


def _get_runner(knn_on: bool) -> _Runner:
    if knn_on not in _RUNNERS:
        _RUNNERS[knn_on] = _Runner(knn_on)
    return _RUNNERS[knn_on]


def kernel(inputs, em, targets, epoch):
    inputs = np.asarray(inputs, dtype=np.float32)
    em = np.asarray(em)
    targets = np.asarray(targets).astype(np.int64)
    epoch_val = int(np.asarray(epoch))
    knn_on = (KNN > 0) and (epoch_val > 4)

    runner = _get_runner(knn_on)
    jax = runner.jax

    # em: cast each core's natural-layout shard to fp8 on the XLA CPU
    # backend (multithreaded, bitwise == ml_dtypes) and start its device
    # transfer immediately — later casts overlap earlier transfers.
    cpu = jax.devices("cpu")[0]
    em_parts = []
    with jax.default_device(cpu):
        for i in range(NCORES):
            q = np.asarray(runner.cast8(em[i * CLOC:(i + 1) * CLOC, :]))
            em_parts.append(jax.device_put(q, runner.devices[i]))

    # pack inp_t/inp_n/tgt_n bytes into one sharded byte tensor
    inp16_n = inputs.astype(np.float16)
    inp16_t = np.ascontiguousarray(inp16_n.T)
    tgt16 = em[targets].astype(np.float16)
    NB = 2 * F * B
    smb = np.empty(3 * NB, dtype=np.uint8)
    smb[0:NB] = inp16_t.view(np.uint8).ravel()
    smb[NB:2 * NB] = inp16_n.view(np.uint8).ravel()
    smb[2 * NB:3 * NB] = tgt16.view(np.uint8).ravel()
    sm = smb.view(runner.f8np).reshape(3 * NB // F, F)
    sm_dev = jax.device_put(sm, runner.sharding)
    em_global = jax.make_array_from_single_device_arrays(
        (C, F), runner.sharding, em_parts)

    loss = runner({"em_n": em_global, "sm_in": sm_dev})
    return np.float32(loss[0, 0])


# revision 22
# speedup vs baseline: 2.4681x; 2.4681x over previous
"""Trainium2 Bass kernel for nn_InvNet_5214090297566 (retrieval_knn).

Strategy (class-sharded tensor parallel over the memory bank, 8 cores):
  - The memory bank em is the only large payload; it ships NATURAL layout
    as fp8_e4m3 (one plain host cast, no host transpose). fp8 is transport
    only: the PE transposes the natural tiles (fp8 transpose mode, step-2
    PSUM out) and the ACT drain converts to fp16, so all matmul arithmetic
    is fp16 x fp16 -> fp32. Quantizing em to fp8 shifts the reference loss
    by only ~2e-5 (measured) — far below tolerance.
  - Feature windows are 128-wide (PE transpose block). A split's tail
    (77/75 features) uses a window ending at the split boundary with the
    input rows above the tail zeroed, so the duplicated em rows contribute
    nothing to the matmul.
  - Target-class rows are gathered on host (pure indexing) and shipped
    replicated; every core computes the full per-split target dots locally
    (no em natural copy on device, no indirect gather, no cross-core sum).
  - Each core: for each 512-class chunk, fp16 matmuls produce per-split
    sims (PSUM) and the full sim (PSUM, accumulated over the same
    feature windows).
  - Packed-key trick: key = fp16(splitdot + 0.5) + (fulldot+0.25)*2^-12*(511/512).
    The fp16 cast quantizes the ranking value; the fulldot payload rides in
    the low mantissa bits (below half the fp16 ulp), so a single max8 pass
    yields top-8 candidates carrying both the ranking value and the full-sim
    value (recovered later by re-casting to fp16 and subtracting).
  - LSE partials via ACT exp+accumulate straight from PSUM (no max needed:
    sim/beta <= ~4 for unit-norm rows).
  - One AllGather of an 81-float-per-row blob (80 keys + 1 sumexp); every
    core redundantly merges (global 6th-largest key per (split,row), masked
    sums, final scalar).
  - Runner: the jax.jit(shard_map(...)) wrapper around the Bass custom call
    is built once and cached; per call, em shards are cast+device_put
    per-device in a pipelined loop (casts overlap in-flight transfers),
    then assembled with make_array_from_single_device_arrays.
"""

import os
os.environ.setdefault("JAX_PLATFORMS", "axon,cpu")

from contextlib import ExitStack

import numpy as np

import concourse.bacc as bacc
import concourse.bass_isa as bass_isa
import concourse.masks as masks
import concourse.mybir as mybir
import concourse.tile as tile

NCORES = 8
B = 256
C = 32768
F = 2048
CLOC = C // NCORES          # 4096 classes per core
NSPLITS = 10
STEP = -(-F // NSPLITS)     # 205
KNN = 6
ISCALE = 20.0               # 1/BETA
NCHUNK = 8                  # class chunks per core
CW = 512                    # chunk width (classes)
NT = 2                      # batch tiles of 128

# fp16 packing constants
HBIAS = 0.5
FQ_SCALE = float((2.0 ** -12) * (511.0 / 512.0))
FQ_OFF = float(0.25 * (2.0 ** -12) * (511.0 / 512.0))
REC_SCALE = float((2.0 ** 12) * (512.0 / 511.0))

# feature windows: per split s, two 128-wide windows
#   W1 = [205s, 205s+128)                 all rows valid (off=0)
#   W2 = [min(205(s+1),F)-128, ...+128)   rows [off:128) valid, off = overlap
# with the input rows [0:off) zeroed so duplicated em rows contribute 0.
WINDOWS = []
for s in range(NSPLITS):
    lo = STEP * s
    hi = min(STEP * (s + 1), F)
    WINDOWS.append((lo, 0))
    w2lo = hi - 128
    WINDOWS.append((w2lo, (lo + 128) - w2lo))

GP_KEY_SPLITS = 8  # splits whose packed key builds on gpsimd (rest on DVE)

F32 = mybir.dt.float32
F16 = mybir.dt.float16
F8 = mybir.dt.float8e4
U8 = mybir.dt.uint8
AF = mybir.ActivationFunctionType
OP = mybir.AluOpType

SW = 81  # stage width: 80 keys + 1 sumexp


def _build(knn_on: bool):
    nc = bacc.Bacc("TRN2", target_bir_lowering=False, debug=False,
                   num_devices=NCORES)

    em_n = nc.dram_tensor("em_n", [CLOC, F], F8, kind="ExternalInput")
    # inp_t/inp_n/tgt_n ship SHARDED as raw bytes (192 fp8-rows per core) and
    # are broadcast on-device by an AllGather, instead of 8x replicated host
    # transfers.  SMROWS rows x F bytes per core; 3 x 1MiB tensors total.
    SMROWS = (2 * F * B * 3) // (F * NCORES)   # 192
    sm_in = nc.dram_tensor("sm_in", [SMROWS, F], U8, kind="ExternalInput")
    out_loss = nc.dram_tensor("loss", [1, 1], F32, kind="ExternalOutput")

    sm_st = nc.dram_tensor("sm_st", [SMROWS, F], U8, kind="Internal")
    sm_g = nc.dram_tensor("sm_g", [NCORES, SMROWS, F], U8, kind="Internal",
                          addr_space="Shared")
    stage = nc.dram_tensor("stage", [NT, 128, SW], F32, kind="Internal")
    gath = nc.dram_tensor("gath", [NCORES, NT, 128, SW], F32, kind="Internal",
                          addr_space="Shared")

    with tile.TileContext(nc) as tc, ExitStack() as ctx:
        singles = ctx.enter_context(tc.tile_pool(name="singles", bufs=1))
        slabs = ctx.enter_context(tc.tile_pool(name="slabs", bufs=2))
        work = ctx.enter_context(tc.tile_pool(name="work", bufs=3))
        keyp = ctx.enter_context(tc.tile_pool(name="keyp", bufs=4))
        big1 = ctx.enter_context(tc.tile_pool(name="big1", bufs=1))
        psum_f = ctx.enter_context(tc.tile_pool(name="psf", bufs=2, space="PSUM"))
        psum_s = ctx.enter_context(tc.tile_pool(name="pss", bufs=3, space="PSUM"))
        psum_t = ctx.enter_context(tc.tile_pool(name="pst", bufs=2, space="PSUM"))

        # ---- broadcast the sharded small tensors, build bitcast views ----
        # (collectives cannot read IO tensors; bounce through Internal dram)
        nc.sync.dma_start(out=sm_st[:, :], in_=sm_in[:, :])
        nc.gpsimd.collective_compute(
            "AllGather", OP.bypass,
            replica_groups=[list(range(NCORES))],
            ins=[sm_st[:, :]], outs=[sm_g[:, :, :]])
        # uint8 transport: the collective ALU canonicalizes fp8 NaN bit
        # patterns (0x79-0x7F/0xF9-0xFF -> 0x7C), which would corrupt raw
        # fp16 bytes shipped as fp8. Integer dtype moves bits untouched.
        # The barrier orders all later DMA reads after the gather.
        tc.strict_bb_all_engine_barrier()
        sm_flat = sm_g[:, :, :].rearrange("a b c -> (a b c)")
        NB = 2 * F * B  # bytes per small tensor
        inp_t = sm_flat[0:NB].bitcast(F16).rearrange("(r c) -> r c", c=B)
        inp_n = sm_flat[NB:2 * NB].bitcast(F16).rearrange("(r c) -> r c", c=F)
        tgt_n = sm_flat[2 * NB:3 * NB].bitcast(F16).rearrange("(r c) -> r c",
                                                              c=F)

        # ---- persistent tiles ----
        id8 = singles.tile([128, 128], F8, tag="id8")
        masks.make_identity(nc, id8[:])

        in_slab = singles.tile([128, 20, B], F16, tag="in_slab")
        nc.vector.memset(in_slab[:], 0.0)
        for j, (wlo, off) in enumerate(WINDOWS):
            nc.sync.dma_start(out=in_slab[off:128, j, :],
                              in_=inp_t[wlo + off:wlo + 128, :])

        cand = {}
        for t in range(NT):
            for s in range(NSPLITS):
                cand[(t, s)] = singles.tile([128, NCHUNK * 8], F32,
                                            tag=f"cand{t}_{s}",
                                            name=f"cand{t}_{s}")
        se_cols = [singles.tile([128, NCHUNK], F32, tag=f"se{t}", name=f"se{t}")
                   for t in range(NT)]
        stage_sb = [singles.tile([128, SW], F32, tag=f"stage{t}", name=f"stg{t}")
                    for t in range(NT)]
        td_loc = [singles.tile([128, NSPLITS], F32, tag=f"td{t}", name=f"td{t}")
                  for t in range(NT)]
        tf_loc = [singles.tile([128, 1], F32, tag=f"tf{t}", name=f"tf{t}")
                  for t in range(NT)]

        # ---- target-class dots (local, from host-gathered rows) ----
        for t in range(NT):
            bsl = slice(t * 128, (t + 1) * 128)
            emt = big1.tile([128, F], F16, tag="emt")
            nc.sync.dma_start(out=emt[:], in_=tgt_n[bsl, :])
            inpn = big1.tile([128, F], F16, tag="inpn")
            nc.sync.dma_start(out=inpn[:], in_=inp_n[bsl, :])
            prod = big1.tile([128, F], F32, tag="prod")
            nc.vector.tensor_tensor(out=prod[:], in0=emt[:], in1=inpn[:],
                                    op=OP.mult)
            nc.vector.reduce_sum(
                out=td_loc[t][:, 0:9],
                in_=prod[:, 0:9 * STEP].rearrange("p (s w) -> p s w", s=9),
                axis=mybir.AxisListType.X)
            nc.vector.reduce_sum(out=td_loc[t][:, 9:10], in_=prod[:, 9 * STEP:F],
                                 axis=mybir.AxisListType.X)
            nc.vector.reduce_sum(out=tf_loc[t][:], in_=td_loc[t][:],
                                 axis=mybir.AxisListType.X)

        # ---- main streaming loops ----
        for c in range(NCHUNK):
            c0 = c * CW
            # natural-layout fp8 slab: 4 class-subtiles of 128 x all features
            nat8 = slabs.tile([128, 4, F], F8, tag="nat8")
            for q in range(4):
                nc.sync.dma_start(out=nat8[:, q, :],
                                  in_=em_n[c0 + q * 128:c0 + (q + 1) * 128, :])
            # PE-transpose each 128x128 block (fp8, step-2 PSUM out), ACT
            # drain converts to fp16 feature-major em_slab
            em_slab = slabs.tile([128, 20, CW], F16, tag="em_slab")
            for j, (wlo, off) in enumerate(WINDOWS):
                for q in range(4):
                    pst = psum_t.tile([128, 128, 2], F8, tag="pst")
                    nc.tensor.transpose(pst[:, :, 0],
                                        nat8[:, q, wlo:wlo + 128], id8[:])
                    nc.scalar.activation(
                        out=em_slab[:, j, q * 128:(q + 1) * 128],
                        in_=pst[:, :, 0], func=AF.Copy, scale=1.0)
            for t in range(NT):
                bsl = slice(t * 128, (t + 1) * 128)
                # full-sim accumulation over all 20 windows (padded rows are 0)
                fbank = psum_f.tile([128, CW], F32, tag="fbank")
                for j in range(20):
                    nc.tensor.matmul(out=fbank[:], lhsT=in_slab[:, j, bsl],
                                     rhs=em_slab[:, j, :],
                                     start=(j == 0), stop=(j == 19))
                # LSE partial: sum(exp(20*fulldot)) for this chunk
                junk = work.tile([128, CW], F32, tag="junk")
                nc.scalar.activation(out=junk[:], in_=fbank[:], func=AF.Exp,
                                     scale=ISCALE,
                                     accum_out=se_cols[t][:, c:c + 1])
                if knn_on:
                    # fq payload from full sim
                    fq = work.tile([128, CW], F32, tag="fq")
                    nc.scalar.activation(out=fq[:], in_=fbank[:], func=AF.Copy,
                                         scale=FQ_SCALE, bias=FQ_OFF)
                    # per-split sims + packed keys + top8
                    for s in range(NSPLITS):
                        sbank = psum_s.tile([128, CW], F32, tag="sbank")
                        for jj in (2 * s, 2 * s + 1):
                            nc.tensor.matmul(out=sbank[:],
                                             lhsT=in_slab[:, jj, bsl],
                                             rhs=em_slab[:, jj, :],
                                             start=(jj == 2 * s),
                                             stop=(jj == 2 * s + 1))
                        h16 = keyp.tile([128, CW], F16, tag="h16")
                        nc.scalar.activation(out=h16[:], in_=sbank[:],
                                             func=AF.Copy, scale=1.0,
                                             bias=HBIAS)
                        key = keyp.tile([128, CW], F32, tag="key")
                        eng = nc.gpsimd if s < GP_KEY_SPLITS else nc.vector
                        eng.tensor_tensor(out=key[:], in0=h16[:], in1=fq[:],
                                          op=OP.add)
                        nc.vector.max(out=cand[(t, s)][:, c * 8:(c + 1) * 8],
                                      in_=key[:])

        # ---- reduce LSE partials + core-level top8 into stage ----
        for t in range(NT):
            nc.vector.reduce_sum(out=stage_sb[t][:, 80:81], in_=se_cols[t][:],
                                 axis=mybir.AxisListType.X)
            if knn_on:
                for s in range(NSPLITS):
                    nc.vector.max(out=stage_sb[t][:, s * 8:(s + 1) * 8],
                                  in_=cand[(t, s)][:])
            nc.sync.dma_start(out=stage[t, :, :], in_=stage_sb[t][:])

        # ---- all-gather ----
        nc.gpsimd.collective_compute(
            "AllGather", OP.bypass,
            replica_groups=[list(range(NCORES))],
            ins=[stage[:, :, :]], outs=[gath[:, :, :, :]])
        tc.strict_bb_all_engine_barrier()

        # ---- final merge (redundant on every core) ----
        rl_tot = singles.tile([128, 1], F32, tag="rl_tot")
        nc.vector.memset(rl_tot[:], 0.0)
        for t in range(NT):
            # global sumexp -> LSE
            se8 = work.tile([128, NCORES], F32, tag="se8")
            nc.sync.dma_start(
                out=se8[:],
                in_=gath[:, t, :, 80:81].rearrange("c p w -> p c w"))
            zt = work.tile([128, 1], F32, tag="zt")
            nc.vector.reduce_sum(out=zt[:], in_=se8[:],
                                 axis=mybir.AxisListType.X)
            lse = work.tile([128, 1], F32, tag="lse")
            nc.scalar.activation(out=lse[:], in_=zt[:], func=AF.Ln)
            tfull = tf_loc[t][:, 0:1]
            # logp_t = 20*fulldot_t - LSE
            logpt = work.tile([128, 1], F32, tag="logpt")
            nc.vector.tensor_scalar(out=logpt[:], in0=tfull, scalar1=ISCALE,
                                    scalar2=None, op0=OP.mult)
            nc.vector.tensor_tensor(out=logpt[:], in0=logpt[:], in1=lse[:],
                                    op=OP.subtract)
            rl = work.tile([128, 1], F32, tag="rl")
            if not knn_on:
                nc.vector.tensor_scalar(out=rl[:], in0=logpt[:], scalar1=-1.0,
                                        scalar2=None, op0=OP.mult)
            else:
                knn_cols = work.tile([128, NSPLITS], F32, tag="knncols")
                cnt_cols = work.tile([128, NSPLITS], F32, tag="cntcols")
                v6_cols = work.tile([128, NSPLITS], F32, tag="v6cols")
                for s in range(NSPLITS):
                    k64 = keyp.tile([128, NCORES * 8], F32, tag="k64")
                    nc.sync.dma_start(
                        out=k64[:],
                        in_=gath[:, t, :, s * 8:(s + 1) * 8]
                        .rearrange("c p w -> p c w"))
                    m8 = work.tile([128, 8], F32, tag="m8")
                    nc.vector.max(out=m8[:], in_=k64[:])
                    nc.vector.tensor_copy(out=v6_cols[:, s:s + 1],
                                          in_=m8[:, 5:6])
                    # decode fulldot payload
                    k16 = work.tile([128, NCORES * 8], F16, tag="k16")
                    nc.vector.tensor_copy(out=k16[:], in_=k64[:])
                    fd = work.tile([128, NCORES * 8], F32, tag="fd")
                    nc.vector.tensor_tensor(out=fd[:], in0=k64[:], in1=k16[:],
                                            op=OP.subtract)
                    nc.vector.tensor_scalar(out=fd[:], in0=fd[:],
                                            scalar1=REC_SCALE, scalar2=-0.25,
                                            op0=OP.mult, op1=OP.add)
                    mask = work.tile([128, NCORES * 8], F32, tag="mask")
                    nc.vector.tensor_scalar(out=mask[:], in0=k64[:],
                                            scalar1=v6_cols[:, s:s + 1],
                                            scalar2=None, op0=OP.is_ge)
                    nc.vector.reduce_sum(out=cnt_cols[:, s:s + 1], in_=mask[:],
                                         axis=mybir.AxisListType.X)
                    nc.vector.scalar_tensor_tensor(
                        out=fd[:], in0=mask[:], scalar=ISCALE, in1=fd[:],
                        op0=OP.mult, op1=OP.mult,
                        accum_out=knn_cols[:, s:s + 1])
                # target keys, same packing construction
                th = work.tile([128, NSPLITS], F32, tag="th")
                nc.vector.tensor_scalar(out=th[:], in0=td_loc[t][:],
                                        scalar1=HBIAS, scalar2=None, op0=OP.add)
                th16 = work.tile([128, NSPLITS], F16, tag="th16")
                nc.vector.tensor_copy(out=th16[:], in_=th[:])
                tfq = work.tile([128, 1], F32, tag="tfq")
                nc.vector.tensor_scalar(out=tfq[:], in0=tfull, scalar1=FQ_SCALE,
                                        scalar2=FQ_OFF, op0=OP.mult, op1=OP.add)
                tkey = work.tile([128, NSPLITS], F32, tag="tkey")
                nc.vector.tensor_scalar(out=tkey[:], in0=th16[:],
                                        scalar1=tfq[:, 0:1], scalar2=None,
                                        op0=OP.add)
                tmask = work.tile([128, NSPLITS], F32, tag="tmask")
                nc.vector.tensor_tensor(out=tmask[:], in0=tkey[:],
                                        in1=v6_cols[:], op=OP.is_ge)
                # rowloss = -10*logp_t - (A - LSE*Cc - logp_t*Tm)/6
                A = work.tile([128, 1], F32, tag="A")
                nc.vector.reduce_sum(out=A[:], in_=knn_cols[:],
                                     axis=mybir.AxisListType.X)
                Cc = work.tile([128, 1], F32, tag="Cc")
                nc.vector.reduce_sum(out=Cc[:], in_=cnt_cols[:],
                                     axis=mybir.AxisListType.X)
                Tm = work.tile([128, 1], F32, tag="Tm")
                nc.vector.reduce_sum(out=Tm[:], in_=tmask[:],
                                     axis=mybir.AxisListType.X)
                u1 = work.tile([128, 1], F32, tag="u1")
                nc.vector.tensor_tensor(out=u1[:], in0=lse[:], in1=Cc[:],
                                        op=OP.mult)
                u2 = work.tile([128, 1], F32, tag="u2")
                nc.vector.tensor_tensor(out=u2[:], in0=logpt[:], in1=Tm[:],
                                        op=OP.mult)
                nc.vector.tensor_tensor(out=A[:], in0=A[:], in1=u1[:],
                                        op=OP.subtract)
                nc.vector.tensor_tensor(out=A[:], in0=A[:], in1=u2[:],
                                        op=OP.subtract)
                nc.vector.tensor_scalar(out=rl[:], in0=logpt[:],
                                        scalar1=-float(NSPLITS), scalar2=None,
                                        op0=OP.mult)
                nc.vector.tensor_scalar(out=A[:], in0=A[:],
                                        scalar1=-1.0 / KNN, scalar2=None,
                                        op0=OP.mult)
                nc.vector.tensor_tensor(out=rl[:], in0=rl[:], in1=A[:],
                                        op=OP.add)
            nc.vector.tensor_tensor(out=rl_tot[:], in0=rl_tot[:], in1=rl[:],
                                    op=OP.add)

        # partition sum -> scalar
        pr = singles.tile([128, 1], F32, tag="pr")
        nc.gpsimd.partition_all_reduce(out_ap=pr[:], in_ap=rl_tot[:],
                                       channels=128,
                                       reduce_op=bass_isa.ReduceOp.add)
        res = singles.tile([1, 1], F32, tag="res")
        denom = float(NSPLITS * B) if knn_on else float(B)
        nc.vector.tensor_scalar(out=res[:], in0=pr[0:1, 0:1],
                                scalar1=1.0 / denom, scalar2=None, op0=OP.mult)
        nc.sync.dma_start(out=out_loss[:, :], in_=res[:])

    nc.finalize()
    return nc


class _Runner:
    """jit(shard_map(bass_exec)) built once; per call, em shards are
    cast+put per-device (pipelined), smalls staged sharded, then invoked."""

    def __init__(self, knn_on: bool):
        import jax
        from jax.sharding import Mesh, NamedSharding, PartitionSpec
        from jax.experimental.shard_map import shard_map
        from concourse.bass2jax import (_bass_exec_p, install_neuronx_cc_hook,
                                        partition_id_tensor)

        self.jax = jax
        import jax.numpy as jnp
        import ml_dtypes
        self.cast8 = jax.jit(lambda a: a.astype(jnp.float8_e4m3))
        self.f8np = ml_dtypes.float8_e4m3
        nc = _build(knn_on)
        install_neuronx_cc_hook()

        partition_name = (nc.partition_id_tensor.name
                          if nc.partition_id_tensor else None)
        in_names, out_names, out_avals = [], [], []
        for alloc in nc.m.functions[0].allocations:
            if not isinstance(alloc, mybir.MemoryLocationSet):
                continue
            name = alloc.memorylocations[0].name
            if alloc.kind == "ExternalInput":
                if name != partition_name:
                    in_names.append(name)
            elif alloc.kind == "ExternalOutput":
                out_names.append(name)
                out_avals.append(jax.core.ShapedArray(
                    tuple(alloc.tensor_shape), mybir.dt.np(alloc.dtype)))
        self.in_names = in_names
        self.out_names = out_names
        self.out_shapes = [(tuple(a.shape), a.dtype) for a in out_avals]
        n_params = len(in_names)
        n_outs = len(out_avals)
        all_in_names = list(in_names) + list(out_names)
        if partition_name is not None:
            all_in_names.append(partition_name)

        def _body(*args):
            operands = list(args)
            if partition_name is not None:
                operands.append(partition_id_tensor())
            return tuple(_bass_exec_p.bind(
                *operands, out_avals=tuple(out_avals),
                in_names=tuple(all_in_names), out_names=tuple(out_names),
                lowering_input_output_aliases=(),
                sim_require_finite=True, sim_require_nnan=True, nc=nc))

        self.devices = list(jax.devices()[:NCORES])
        mesh = Mesh(np.asarray(self.devices), ("core",))
        self.sharding = NamedSharding(mesh, PartitionSpec("core"))
        in_specs = (PartitionSpec("core"),) * (n_params + n_outs)
        out_specs = (PartitionSpec("core"),) * n_outs
        self.fn = jax.jit(
            shard_map(_body, mesh=mesh, in_specs=in_specs,
                      out_specs=out_specs, check_rep=False),
            donate_argnums=tuple(range(n_params, n_params + n_outs)),
            keep_unused=True)

    def __call__(self, dev_inputs: dict):
        jax = self.jax
        zeros = [np.zeros((NCORES * s[0], *s[1:]), d)
                 for (s, d) in self.out_shapes]
        out = self.fn(*[dev_inputs[n] for n in self.in_names], *zeros)
        jax.block_until_ready(out)
        i = self.out_names.index("loss")
        return np.asarray(out[i]).reshape(NCORES, 1, 1)[0]


_RUNNERS = {}


def _get_runner(knn_on: bool) -> _Runner:
    if knn_on not in _RUNNERS:
        _RUNNERS[knn_on] = _Runner(knn_on)
    return _RUNNERS[knn_on]


def kernel(inputs, em, targets, epoch):
    inputs = np.asarray(inputs, dtype=np.float32)
    em = np.asarray(em)
    targets = np.asarray(targets).astype(np.int64)
    epoch_val = int(np.asarray(epoch))
    knn_on = (KNN > 0) and (epoch_val > 4)

    runner = _get_runner(knn_on)
    jax = runner.jax

    # em: cast each core's natural-layout shard to fp8 on the XLA CPU
    # backend (multithreaded, bitwise == ml_dtypes) and start its device
    # transfer immediately — later casts overlap earlier transfers.
    cpu = jax.devices("cpu")[0]
    em_parts = []
    with jax.default_device(cpu):
        for i in range(NCORES):
            q = np.asarray(runner.cast8(em[i * CLOC:(i + 1) * CLOC, :]))
            em_parts.append(jax.device_put(q, runner.devices[i]))

    # pack inp_t/inp_n/tgt_n bytes into one sharded byte tensor
    inp16_n = inputs.astype(np.float16)
    inp16_t = np.ascontiguousarray(inp16_n.T)
    tgt16 = em[targets].astype(np.float16)
    NB = 2 * F * B
    smb = np.empty(3 * NB, dtype=np.uint8)
    smb[0:NB] = inp16_t.view(np.uint8).ravel()
    smb[NB:2 * NB] = inp16_n.view(np.uint8).ravel()
    smb[2 * NB:3 * NB] = tgt16.view(np.uint8).ravel()
    sm = smb.reshape(3 * NB // F, F)
    sm_dev = jax.device_put(sm, runner.sharding)
    em_global = jax.make_array_from_single_device_arrays(
        (C, F), runner.sharding, em_parts)

    loss = runner({"em_n": em_global, "sm_in": sm_dev})
    return np.float32(loss[0, 0])
